# revision 1
# baseline (speedup 1.0000x reference)
"""Causal self-attention (RoPE) Trainium2 kernel, 8-core SPMD.

Sharding:
  Launch A: tensor-parallel over heads — core i computes heads (2i, 2i+1):
    qkv projection slice, RoPE, causal flash-style attention -> yT [128, 4096].
  Host: reshuffle yT col-shards (heads) -> row-shards (tokens).
  Launch B: data-parallel over tokens — core i projects its 512 token rows
    through the full w_proj (+bias) -> out [512, 1024].

Layout notes:
  - Everything on-chip is kept "transposed": features on partitions, tokens on
    the free axis, which makes every matmul a natural lhsT/rhs pair.
  - Head dims are permuted to [evens | odds] so RoPE's rotate-half becomes a
    32-partition block swap (done with tiny SBUF->SBUF DMAs). The permutation
    is applied to both q and k weight columns, leaving scores unchanged.
  - Scores are computed transposed ([k, q]) so that the AV matmul can consume
    probs directly; softmax denominators come from a ones-row appended to v
    (lhsT = [v | 1]), making the row-sum free on the PE.
  - exp(score) is computed without max-subtraction (scores ~ N(0,1) after the
    1/sqrt(D) scale; fp32 exp is safe) — mathematically identical to the
    reference softmax.
"""

import numpy as np
from contextlib import ExitStack

import concourse.bass as bass
import concourse.tile as tile
from concourse import bacc, mybir
from concourse.bass_utils import run_bass_kernel_spmd
from concourse.masks import make_identity

F32 = mybir.dt.float32
F32R = mybir.dt.float32r
BF16 = mybir.dt.bfloat16

B, T, C = 2, 2048, 1024
H, D = 16, 64
NCORES = 8
HPC = H // NCORES          # heads per core (2)
BT = B * T                 # 4096 token rows
TCHUNK = 512               # token chunk (matmul moving size)
NKT_C = C // 128           # k-tiles over the C contraction (8)
NTCH = BT // TCHUNK        # token chunks (8)
NEG = -1.0e9


def _r(ap, dt=F32R):
    return ap.bitcast(dt)


def build_launch_a():
    nc = bacc.Bacc("TRN2", target_bir_lowering=False, debug=False,
                   num_devices=NCORES)
    xT_d = nc.dram_tensor("xT", [C, BT], F32R, kind="ExternalInput").ap()
    wq_d = nc.dram_tensor("wq", [C, 128], F32R, kind="ExternalInput").ap()
    wk_d = nc.dram_tensor("wk", [C, 128], F32R, kind="ExternalInput").ap()
    wv_d = nc.dram_tensor("wv", [C, 128], F32R, kind="ExternalInput").ap()
    cos_d = nc.dram_tensor("cosT", [128, T], F32, kind="ExternalInput").ap()
    sin_d = nc.dram_tensor("sinT", [128, T], F32, kind="ExternalInput").ap()
    mask_d = nc.dram_tensor("mask", [128, 128], F32, kind="ExternalInput").ap()
    y_d = nc.dram_tensor("y", [128, BT], F32, kind="ExternalOutput").ap()
    snd_d = nc.dram_tensor("snd", [HPC, BT], F32, kind="ExternalOutput").ap()

    with tile.TileContext(nc) as tc, ExitStack() as ctx:
        consts = ctx.enter_context(tc.tile_pool(name="consts", bufs=1))
        persist = ctx.enter_context(tc.tile_pool(name="persist", bufs=1))
        xin = ctx.enter_context(tc.tile_pool(name="xin", bufs=2))
        work = ctx.enter_context(tc.tile_pool(name="work", bufs=3))
        probp = ctx.enter_context(tc.tile_pool(name="probp", bufs=4))
        smalls = ctx.enter_context(tc.tile_pool(name="smalls", bufs=4))
        psproj = ctx.enter_context(tc.tile_pool(name="psproj", bufs=2, space="PSUM"))
        pstr = ctx.enter_context(tc.tile_pool(name="pstr", bufs=2, space="PSUM"))
        psscore = ctx.enter_context(tc.tile_pool(name="psscore", bufs=2, space="PSUM"))
        psy = ctx.enter_context(tc.tile_pool(name="psy", bufs=1, space="PSUM"))

        # --- constants ---
        wq_t = consts.tile([128, NKT_C, 128], F32R, tag="wq")
        wk_t = consts.tile([128, NKT_C, 128], F32R, tag="wk")
        wv_t = consts.tile([128, NKT_C, 128], F32R, tag="wv")
        nc.sync.dma_start(out=wq_t[:], in_=wq_d.rearrange("(kt p) c -> p kt c", p=128))
        nc.sync.dma_start(out=wk_t[:], in_=wk_d.rearrange("(kt p) c -> p kt c", p=128))
        nc.sync.dma_start(out=wv_t[:], in_=wv_d.rearrange("(kt p) c -> p kt c", p=128))
        cos_t = consts.tile([128, T], F32, tag="cos")
        sin_t = consts.tile([128, T], F32, tag="sin")
        nc.sync.dma_start(out=cos_t[:], in_=cos_d)
        nc.sync.dma_start(out=sin_t[:], in_=sin_d)
        mask_t = consts.tile([128, 128], F32, tag="mask")
        nc.sync.dma_start(out=mask_t[:], in_=mask_d)
        ident = consts.tile([128, 128], BF16, tag="ident")
        make_identity(nc, ident)

        # --- persistent intermediates ---
        qrot = persist.tile([128, BT], F32R, tag="qrot")
        krot = persist.tile([128, BT], F32R, tag="krot")
        # v in natural [token, dim] layout per global 128-token tile, columns
        # [h0 dims | ones | h1 dims | ones] so each head's AV lhsT is a
        # contiguous [128, 65] slice whose last column computes the softmax
        # denominator for free.
        vnat = persist.tile([128, BT // 128, 130], BF16, tag="vnat")
        nc.gpsimd.memset(vnat[:, :, 64:65], 1.0)
        nc.gpsimd.memset(vnat[:, :, 129:130], 1.0)

        # =================== phase 1: projections + RoPE ===================
        for tch in range(NTCH):
            g0 = tch * TCHUNK
            bb, t0 = divmod(g0, T)
            xt = xin.tile([128, NKT_C, TCHUNK], F32R, tag="xt")
            nc.sync.dma_start(
                out=xt[:],
                in_=xT_d.rearrange("(kt p) t -> p kt t", p=128)[:, :, g0:g0 + TCHUNK])

            for name, wt in (("q", wq_t), ("k", wk_t), ("v", wv_t)):
                ps = psproj.tile([128, TCHUNK], F32, tag="psproj")
                for kt in range(NKT_C):
                    nc.tensor.matmul(ps[:], wt[:, kt, :], xt[:, kt, :],
                                     start=(kt == 0), stop=(kt == NKT_C - 1))
                if name == "v":
                    vstage = work.tile([128, TCHUNK], BF16, tag="vstage")
                    nc.scalar.copy(vstage[:], ps[:])
                    # PE-transpose each [128, 128] block: rows become tokens,
                    # cols become [h0 dims | h1 dims].
                    for j in range(TCHUNK // 128):
                        ktg = (g0 + j * 128) // 128
                        pst = pstr.tile([128, 128], BF16, tag="pstr")
                        nc.tensor.transpose(
                            pst[:], vstage[:, 128 * j:128 * (j + 1)], ident[:])
                        nc.vector.tensor_copy(vnat[:, ktg, 0:64], pst[:, 0:64])
                        nc.vector.tensor_copy(vnat[:, ktg, 65:129],
                                              pst[:, 64:128])
                else:
                    raw = work.tile([128, TCHUNK], F32, tag="raw")
                    nc.scalar.copy(raw[:], ps[:])
                    # rotate-half: swap 32-row blocks within each head
                    sh = work.tile([128, TCHUNK], F32, tag="sh")
                    for blk in range(4):
                        src = blk ^ 1
                        nc.sync.dma_start(out=sh[32 * blk:32 * (blk + 1), :],
                                          in_=raw[32 * src:32 * (src + 1), :])
                    dest = qrot if name == "q" else krot
                    tmp = work.tile([128, TCHUNK], F32, tag="ropetmp")
                    cslice = cos_t[:, t0:t0 + TCHUNK]
                    sslice = sin_t[:, t0:t0 + TCHUNK]
                    nc.vector.tensor_mul(dest[:, g0:g0 + TCHUNK], raw[:], cslice)
                    nc.vector.tensor_mul(tmp[:], sh[:], sslice)
                    nc.vector.tensor_add(dest[:, g0:g0 + TCHUNK],
                                         dest[:, g0:g0 + TCHUNK], tmp[:])

        # =================== phase 2: attention ===================
        for bb in range(B):
            for qc in range(T // TCHUNK):
                q0 = qc * TCHUNK
                gq = bb * T + q0
                nkt = (q0 + TCHUNK) // 128
                ys = [psy.tile([65, TCHUNK], F32, tag=f"psy{h}", name=f"psy{h}")
                      for h in range(HPC)]
                for kt in range(nkt):
                    k0 = kt * 128
                    j0 = k0 - q0
                    js = max(j0, 0)   # columns < js are fully masked: skip them
                    ktg = (bb * T + k0) // 128
                    for h in range(HPC):
                        hp = 64 * h
                        pss = psscore.tile([128, TCHUNK], F32, tag="pss")
                        nc.tensor.matmul(
                            pss[:, js:TCHUNK],
                            krot[hp:hp + 64, bb * T + k0:bb * T + k0 + 128],
                            qrot[hp:hp + 64, gq + js:gq + TCHUNK],
                            start=True, stop=True, tile_position=(hp, 0))
                        probs = probp.tile([128, TCHUNK], BF16, tag=f"probs{h}")
                        if j0 >= 0:
                            # straddles the causal diagonal
                            nc.vector.tensor_add(pss[:, j0:j0 + 128],
                                                 pss[:, j0:j0 + 128], mask_t[:])
                        nc.scalar.activation(
                            probs[:, js:TCHUNK], pss[:, js:TCHUNK],
                            mybir.ActivationFunctionType.Exp,
                            scale=float(1.0 / np.sqrt(D)))
                        nc.tensor.matmul(ys[h][:, js:TCHUNK],
                                         vnat[:, ktg, 65 * h:65 * h + 65],
                                         probs[:, js:TCHUNK],
                                         start=(kt == 0), stop=(kt == nkt - 1))
                # epilogue: ship unnormalized y + denominators (row 64)
                for h in range(HPC):
                    yts = work.tile([65, TCHUNK], F32, tag="yts")
                    nc.vector.tensor_copy(yts[:], ys[h][:])
                    nc.sync.dma_start(out=y_d[64 * h:64 * (h + 1), gq:gq + TCHUNK],
                                      in_=yts[0:64, :])
                    nc.sync.dma_start(out=snd_d[h, gq:gq + TCHUNK],
                                      in_=yts[64:65, :])

    nc.compile()
    return nc


def build_launch_b():
    nc = bacc.Bacc("TRN2", target_bir_lowering=False, debug=False,
                   num_devices=NCORES)
    TLOC = BT // NCORES  # 512 tokens per core
    yT_d = nc.dram_tensor("yT", [C, TLOC], F32R, kind="ExternalInput").ap()
    wp_d = nc.dram_tensor("wp", [C, C], F32R, kind="ExternalInput").ap()
    bias_d = nc.dram_tensor("biasb", [128, C], F32, kind="ExternalInput").ap()
    s_d = nc.dram_tensor("sums", [H, TLOC], F32, kind="ExternalInput").ap()
    out_d = nc.dram_tensor("out", [TLOC, C], F32, kind="ExternalOutput").ap()

    with tile.TileContext(nc) as tc, ExitStack() as ctx:
        consts = ctx.enter_context(tc.tile_pool(name="consts", bufs=1))
        work = ctx.enter_context(tc.tile_pool(name="work", bufs=3))
        pspool = ctx.enter_context(tc.tile_pool(name="ps", bufs=4, space="PSUM"))

        wp_t = consts.tile([128, NKT_C, C], F32R, tag="wp")
        nc.sync.dma_start(out=wp_t[:], in_=wp_d.rearrange("(kt p) c -> p kt c", p=128))
        bias_t = consts.tile([128, C], F32, tag="bias")
        nc.sync.dma_start(out=bias_t[:], in_=bias_d)
        yT_t = consts.tile([128, NKT_C, TLOC], F32R, tag="yT")
        nc.sync.dma_start(out=yT_t[:], in_=yT_d.rearrange("(kt p) t -> p kt t", p=128))
        # normalize: yT[hd, t] *= 1 / sums[head(hd), t]
        s_t = consts.tile([H, TLOC], F32, tag="sums")
        nc.sync.dma_start(out=s_t[:], in_=s_d)
        rec_t = consts.tile([H, TLOC], F32, tag="rec")
        nc.vector.reciprocal(rec_t[:], s_t[:])
        rec_dram = nc.dram_tensor("rec_scratch", [H, TLOC], F32)
        nc.sync.dma_start(out=rec_dram.ap(), in_=rec_t[:])
        scale_t = consts.tile([128, NKT_C, TLOC], F32, tag="scale")
        for kt in range(NKT_C):
            for hh in range(2):
                row = rec_dram.ap()[2 * kt + hh:2 * kt + hh + 1, :]
                bcast_src = bass.AP(tensor=row.tensor, offset=row.offset,
                                    ap=[[0, 64]] + list(row.ap)[1:])
                nc.sync.dma_start(
                    out=scale_t[64 * hh:64 * (hh + 1), kt, :], in_=bcast_src)
        nc.vector.tensor_mul(yT_t[:], yT_t[:], scale_t[:])

        for mt in range(TLOC // 128):        # 4 output row tiles (tokens)
            for nch in range(C // TCHUNK):   # 2 output col chunks
                ps = pspool.tile([128, TCHUNK], F32, tag="ps")
                for kt in range(NKT_C):
                    nc.tensor.matmul(
                        ps[:],
                        yT_t[:, kt, 128 * mt:128 * (mt + 1)],
                        wp_t[:, kt, TCHUNK * nch:TCHUNK * (nch + 1)],
                        start=(kt == 0), stop=(kt == NKT_C - 1))
                ot = work.tile([128, TCHUNK], F32, tag="ot")
                nc.vector.tensor_add(ot[:], ps[:],
                                     bias_t[:, TCHUNK * nch:TCHUNK * (nch + 1)])
                nc.sync.dma_start(
                    out=out_d[128 * mt:128 * (mt + 1),
                              TCHUNK * nch:TCHUNK * (nch + 1)],
                    in_=ot[:])

    nc.compile()
    return nc


def _host_prep(x, w_qkv):
    xT = np.ascontiguousarray(x.reshape(BT, C).T)  # [C, BT]
    perm = np.concatenate([np.arange(0, D, 2), np.arange(1, D, 2)])
    inv = 1.0 / (10000.0 ** (np.arange(0, D, 2, dtype=np.float64) / D))  # [32]
    f = np.outer(np.arange(T, dtype=np.float64), inv)  # [T, 32]
    cosT = np.cos(f).T.astype(np.float32)  # [32, T]
    sinT = np.sin(f).T.astype(np.float32)
    c64 = np.concatenate([cosT, cosT], 0)
    s64 = np.concatenate([-sinT, sinT], 0)
    C128 = np.ascontiguousarray(np.concatenate([c64, c64], 0))  # [128, T]
    S128 = np.ascontiguousarray(np.concatenate([s64, s64], 0))
    p = np.arange(128)
    mask = np.where(p[:, None] <= p[None, :], 0.0, NEG).astype(np.float32)

    in_maps = []
    for i in range(NCORES):
        h0, h1 = HPC * i, HPC * i + 1
        wq = np.concatenate([w_qkv[:, h0 * D + perm], w_qkv[:, h1 * D + perm]], 1)
        wk = np.concatenate([w_qkv[:, C + h0 * D + perm], w_qkv[:, C + h1 * D + perm]], 1)
        wv = np.concatenate([w_qkv[:, 2 * C + h0 * D:2 * C + (h0 + 1) * D],
                             w_qkv[:, 2 * C + h1 * D:2 * C + (h1 + 1) * D]], 1)
        in_maps.append({
            "xT": xT,
            "wq": np.ascontiguousarray(wq),
            "wk": np.ascontiguousarray(wk),
            "wv": np.ascontiguousarray(wv),
            "cosT": C128, "sinT": S128, "mask": mask,
        })
    return in_maps


_CACHE = {}


def _get_kernels():
    if "a" not in _CACHE:
        _CACHE["a"] = build_launch_a()
        _CACHE["b"] = build_launch_b()
    return _CACHE["a"], _CACHE["b"]


def run(x, w_qkv, w_proj, b_proj, trace=False, tmpdirs=(None, None)):
    nca, ncb = _get_kernels()
    in_maps_a = _host_prep(np.asarray(x), np.asarray(w_qkv))
    res_a = run_bass_kernel_spmd(nca, in_maps_a, list(range(NCORES)),
                                 trace=trace, tmpdir=tmpdirs[0])
    # yT_full[h*64+d, token] — assemble, then shard by token chunk
    yT = np.concatenate([res_a.results[i]["y"] for i in range(NCORES)], 0)  # [1024, 4096]
    sums = np.concatenate([res_a.results[i]["snd"] for i in range(NCORES)], 0)  # [16, 4096]
    wp = np.asarray(w_proj)
    bias_b = np.broadcast_to(np.asarray(b_proj), (128, C))
    TLOC = BT // NCORES
    in_maps_b = [{
        "yT": np.ascontiguousarray(yT[:, i * TLOC:(i + 1) * TLOC]),
        "wp": wp,
        "biasb": np.ascontiguousarray(bias_b),
        "sums": np.ascontiguousarray(sums[:, i * TLOC:(i + 1) * TLOC]),
    } for i in range(NCORES)]
    res_b = run_bass_kernel_spmd(ncb, in_maps_b, list(range(NCORES)),
                                 trace=trace, tmpdir=tmpdirs[1])
    out = np.concatenate([res_b.results[i]["out"] for i in range(NCORES)], 0)
    return out.reshape(B, T, C).astype(np.float32), res_a, res_b


def kernel(x, w_qkv, w_proj, b_proj):
    out, _, _ = run(x, w_qkv, w_proj, b_proj)
    return out



# revision 23
# speedup vs baseline: 1.4266x; 1.4266x over previous
"""Causal self-attention (RoPE) Trainium2 kernel, 8-core SPMD, single launch.

Sharding: tensor-parallel over heads. Core i owns heads (2i, 2i+1):
  - qkv projection slice (bf16), RoPE, causal attention, and a PARTIAL output
    projection over its own 128 y-features. Host sums the 8 partials and adds
    the bias (both linear, so they commute with the per-core split).

Design notes:
  - All matmuls are bf16 (fp32r runs in 4-cycle/row HIGH mode on HW).
  - q/k kept feature-major [dim, tok]; head dims permuted to [evens|odds] so
    RoPE rotate-half becomes a signed 32-row block swap, done on the PE with a
    small permutation matmul (PT) instead of SBUF->SBUF DMAs.
  - v computed directly token-major [tok, dim] (x-tile as lhsT), with a ones
    column appended per head so the AV matmul also produces softmax
    denominators for free (row 64 of ys).
  - Scores are [k, q] with the two heads' 512-wide strips packed side by side
    in one 2-bank PSUM strip -> ONE wide exp instruction per (kt, window) on
    the Activation engine (its throughput is the second roofline after PE).
  - Causal masking of diagonal blocks: Pool-engine affine_select zeroing the
    upper triangle of probs (exp is computed unmasked; scores ~ N(0,1)).
  - Normalization: reciprocal of the denominator rows, broadcast across the
    128 feature partitions with a tiny PE matmul (pat2), applied to y during
    the PSUM->SBUF copy, BEFORE the output projection (scale commutes).
  - Projection/out-projection matmuls are interleaved into the attention kt
    loop as "filler" so the PE never head-of-line blocks on exp.
"""

import math
from collections import deque

import numpy as np
import ml_dtypes

import concourse.bass as bass
import concourse.tile as tile
from concourse import bacc, mybir
from concourse.bass_utils import run_bass_kernel_spmd

F32 = mybir.dt.float32
BF16 = mybir.dt.bfloat16

B, T, C = 2, 2048, 1024
H, D = 16, 64
NCORES = 8
HPC = H // NCORES          # heads per core (2)
BT = B * T                 # 4096 tokens
TCH = 512                  # token chunk = q window
NKT = C // 128             # contraction tiles over C (8)
NCH = BT // TCH            # chunks (8)
KPW = TCH // 128           # k-tiles per window step (4)


def build_kernel(debug=False):
    nc = bacc.Bacc("TRN2", target_bir_lowering=False, debug=False,
                   num_devices=NCORES)
    xT_d = nc.dram_tensor("xT", [C, BT], BF16, kind="ExternalInput").ap()
    wq_d = nc.dram_tensor("wq", [C, 128], BF16, kind="ExternalInput").ap()
    wk_d = nc.dram_tensor("wk", [C, 128], BF16, kind="ExternalInput").ap()
    wv_d = nc.dram_tensor("wv", [C, 128], BF16, kind="ExternalInput").ap()
    cos_d = nc.dram_tensor("cosT", [128, T], BF16, kind="ExternalInput").ap()
    sin_d = nc.dram_tensor("sinT", [128, T], BF16, kind="ExternalInput").ap()
    pt_d = nc.dram_tensor("PT", [128, 128], BF16, kind="ExternalInput").ap()
    pat_d = nc.dram_tensor("pat33", [33, 128], BF16, kind="ExternalInput").ap()
    wp_d = nc.dram_tensor("wp", [128, C], BF16, kind="ExternalInput").ap()
    out_d = nc.dram_tensor("partial", [BT, C], BF16, kind="ExternalOutput").ap()
    if debug:
        qrot_d = nc.dram_tensor("qrot_d", [128, BT], BF16, kind="ExternalOutput").ap()
        krot_d = nc.dram_tensor("krot_d", [128, BT], BF16, kind="ExternalOutput").ap()
        vnat_d = nc.dram_tensor("vnat_d", [128, BT // 128, HPC, 65], BF16,
                                kind="ExternalOutput").ap()
        ysb_d = nc.dram_tensor("ysb_d", [NCH, 128, TCH], BF16, kind="ExternalOutput").ap()
        rr_d = nc.dram_tensor("rr_d", [NCH, HPC, TCH], BF16, kind="ExternalOutput").ap()
        pr_d = nc.dram_tensor("pr_d", [16, 128, 2, TCH], BF16, kind="ExternalOutput").ap()
        brec_d = nc.dram_tensor("brec_d", [NCH, 128, TCH], BF16, kind="ExternalOutput").ap()

    with tile.TileContext(nc) as tc:
        with tc.tile_pool(name="consts", bufs=1) as consts, \
             tc.tile_pool(name="persist", bufs=1) as persist, \
             tc.tile_pool(name="xinp", bufs=2) as xinp, \
             tc.tile_pool(name="work", bufs=2) as work, \
             tc.tile_pool(name="probsp", bufs=3) as probsp, \
             tc.tile_pool(name="ps", bufs=1, space="PSUM") as ps:

            # ---------------- constants ----------------
            wq_t = consts.tile([128, NKT, 128], BF16, tag="wq")
            wk_t = consts.tile([128, NKT, 128], BF16, tag="wk")
            wv_t = consts.tile([128, NKT, 128], BF16, tag="wv")
            nc.sync.dma_start(out=wq_t[:], in_=wq_d.rearrange("(kt p) c -> p kt c", p=128))
            nc.sync.dma_start(out=wk_t[:], in_=wk_d.rearrange("(kt p) c -> p kt c", p=128))
            nc.sync.dma_start(out=wv_t[:], in_=wv_d.rearrange("(kt p) c -> p kt c", p=128))
            cos_t = consts.tile([128, T], BF16, tag="cos")
            sin_t = consts.tile([128, T], BF16, tag="sin")
            nc.sync.dma_start(out=cos_t[:], in_=cos_d)
            nc.sync.dma_start(out=sin_t[:], in_=sin_d)
            pt_t = consts.tile([128, 128], BF16, tag="pt")
            nc.sync.dma_start(out=pt_t[:], in_=pt_d)
            pat_t = consts.tile([33, 128], BF16, tag="pat")
            nc.sync.dma_start(out=pat_t[:], in_=pat_d)
            wp_t = consts.tile([128, C], BF16, tag="wp")
            nc.sync.dma_start(out=wp_t[:], in_=wp_d)

            # ---------------- persistent ----------------
            qrot = persist.tile([128, BT], BF16, tag="qrot")
            krot = persist.tile([128, BT], BF16, tag="krot")
            # v token-major: [tok, ktile, head, dim|one]
            vnat = persist.tile([128, BT // 128, HPC, 65], BF16, tag="vnat")
            nc.gpsimd.memset(vnat[:, :, :, 64:65], 1.0)
            # denominator reciprocals land on rows 0 and 32 (32-aligned engine
            # bases); other rows stay zero so the pat33 broadcast matmul
            # (contraction 33) reproduces each half from its row.
            rT = persist.tile([33, TCH], BF16, tag="rT")
            nc.gpsimd.memset(rT[:, :], 0.0)

            scale = float(1.0 / math.sqrt(D))

            # ---------------- helpers ----------------
            def proj_ops(c):
                """Filler closures computing qkv projection + RoPE of chunk c."""
                g0 = c * TCH
                t0 = g0 % T
                ops = []

                def dma_x():
                    xt = xinp.tile([128, NKT, TCH], BF16, tag="xt", name=f"xt{c}")
                    nc.sync.dma_start(
                        out=xt[:],
                        in_=xT_d.rearrange("(kt p) t -> p kt t", p=128)[:, :, g0:g0 + TCH])
                    return xt
                xt_box = {}
                ops.append(lambda: xt_box.__setitem__("t", dma_x()))

                def qk_mm(wt, half, box, tag):
                    def f():
                        if half == 0:
                            box["ps"] = ps.tile([128, TCH], F32, tag="gen", bufs=2,
                                                name=f"{tag}ps{c}")
                        pp = box["ps"]
                        for kt in range(4 * half, 4 * half + 4):
                            nc.tensor.matmul(pp[:], wt[:, kt, :], xt_box["t"][:, kt, :],
                                             start=(kt == 0), stop=(kt == 7))
                    return f

                def qk_rope(box, dest, tag):
                    def f():
                        raw = work.tile([128, TCH], BF16, tag="raw", name=f"raw{tag}{c}")
                        nc.scalar.copy(raw[:], box["ps"][:])
                        sh = ps.tile([128, TCH], F32, tag="gen", bufs=2, name=f"sh{tag}{c}")
                        nc.tensor.matmul(sh[:], pt_t[:], raw[:], start=True, stop=True)
                        tmp = work.tile([128, TCH], BF16, tag="tmp", name=f"tmp{tag}{c}")
                        nc.vector.tensor_mul(tmp[:], sh[:], sin_t[:, t0:t0 + TCH])
                        dst = dest[:, g0:g0 + TCH]
                        nc.vector.tensor_mul(dst, raw[:], cos_t[:, t0:t0 + TCH])
                        nc.vector.tensor_add(dst, dst, tmp[:])
                    return f

                # NOTE: every "gen"-ring PSUM tile must be consumed before two
                # more gen allocations happen (ring bufs=2) — keep each
                # producer's reader within the next closure.
                qb, kb = {}, {}
                ops.append(qk_mm(wq_t, 0, qb, "q"))
                ops.append(qk_mm(wq_t, 1, qb, "q"))
                ops.append(qk_rope(qb, qrot, "q"))
                ops.append(qk_mm(wk_t, 0, kb, "k"))
                ops.append(qk_mm(wk_t, 1, kb, "k"))

                def v_tile(m):
                    def f():
                        vv = ps.tile([128, HPC, 64], F32, tag="gen", bufs=2,
                                     name=f"vv{c}_{m}")
                        for kt in range(NKT):
                            nc.tensor.matmul(vv[:, :, :],
                                             xt_box["t"][:, kt, 128 * m:128 * (m + 1)],
                                             wv_t[:, kt, :],
                                             start=(kt == 0), stop=(kt == NKT - 1))
                        vt = c * KPW + m
                        nc.vector.tensor_copy(vnat[:, vt, :, 0:64], vv[:, :, :])
                    return f

                ops.append(qk_rope(kb, krot, "k"))
                ops.append(v_tile(0))
                ops.append(v_tile(1))
                ops.append(v_tile(2))
                ops.append(v_tile(3))
                return ops

            def outproj_ops(c, ysb):
                """Filler closures projecting normalized y chunk c through wp."""
                g0 = c * TCH
                ops = []

                def otile(m):
                    def f():
                        for n in range(2):
                            po = ps.tile([128, TCH], F32, tag="gen", bufs=2,
                                         name=f"po{c}_{m}_{n}")
                            nc.tensor.matmul(po[:], ysb[:, 128 * m:128 * (m + 1)],
                                             wp_t[:, TCH * n:TCH * (n + 1)],
                                             start=True, stop=True)
                            ostage = work.tile([128, TCH], BF16, tag="ostage",
                                               bufs=3, name=f"os{c}_{m}_{n}")
                            if (2 * m + n) % 4 == 3:
                                nc.scalar.copy(ostage[:], po[:])
                            else:
                                nc.vector.tensor_copy(ostage[:], po[:])
                            nc.sync.dma_start(
                                out=out_d[g0 + 128 * m:g0 + 128 * (m + 1),
                                          TCH * n:TCH * (n + 1)],
                                in_=ostage[:])
                    return f
                for m in range(4):
                    ops.append(otile(m))
                return ops

            # ---------------- main pipeline ----------------
            queue = deque(proj_ops(0))
            while queue:  # chunk 0 projection up-front
                queue.popleft()()

            prev_outproj = []
            for c in range(NCH):
                b, wl = divmod(c, T // TCH)
                q0l = wl * TCH
                gq = c * TCH
                nkt = KPW * (wl + 1)

                if c + 1 < NCH:
                    queue.extend(proj_ops(c + 1))
                queue.extend(prev_outproj)
                prev_outproj = []

                ys = [ps.tile([65, TCH], F32, tag=f"ys{h}", bufs=1, name=f"ys{h}_{c}")
                      for h in range(HPC)]
                strips = {}
                probs = {}

                def sc(kt):
                    k0l = 128 * kt
                    js = max(k0l - q0l, 0)
                    st = ps.tile([128, 2, TCH], F32, tag="strip", bufs=2,
                                 name=f"st{c}_{kt}")
                    strips[kt] = st
                    for h in range(HPC):
                        hp = 64 * h
                        nc.tensor.matmul(
                            st[:, h, js:TCH],
                            krot[hp:hp + 64, b * T + k0l:b * T + k0l + 128],
                            qrot[hp:hp + 64, gq + js:gq + TCH],
                            start=True, stop=True, tile_position=(hp, 0))
                    pr = probsp.tile([128, 2, TCH], BF16, tag="pr", name=f"pr{c}_{kt}")
                    probs[kt] = pr
                    nc.scalar.activation(pr[:, :, js:TCH], st[:, :, js:TCH],
                                         mybir.ActivationFunctionType.Exp,
                                         scale=scale)
                    if js > 0 or kt == q0l // 128:
                        # diagonal block: zero probs where q < k, i.e. keep
                        # j - p >= 0 (p = key partition, j = query column)
                        nc.gpsimd.affine_select(
                            out=pr[:, :, js:js + 128], in_=pr[:, :, js:js + 128],
                            compare_op=mybir.AluOpType.is_ge, fill=0.0,
                            base=0, pattern=[[0, 2], [1, 128]],
                            channel_multiplier=-1)
                    if debug and c == NCH - 1:
                        nc.sync.dma_start(out=pr_d[kt], in_=pr[:, :, :])

                def av(kt):
                    k0l = 128 * kt
                    js = max(k0l - q0l, 0)
                    vt = b * (T // 128) + kt
                    for h in range(HPC):
                        nc.tensor.matmul(ys[h][:, js:TCH],
                                         vnat[:, vt, h, :],
                                         probs[kt][:, h, js:TCH],
                                         start=(kt == 0), stop=(kt == nkt - 1))
                    del probs[kt], strips[kt]

                sc(0)
                if nkt > 1:
                    sc(1)
                for kt in range(nkt):
                    av(kt)
                    ndrain = -(-len(queue) // (nkt - kt)) if kt < nkt else 0
                    for _ in range(min(ndrain, len(queue))):
                        queue.popleft()()
                    if kt + 2 < nkt:
                        sc(kt + 2)

                # normalization: ysb = ys/denom (bf16), feature-major
                with nc.allow_low_precision(reason="denominators ~O(1..2k), bf16 recip ok"):
                    for h in range(HPC):
                        nc.vector.reciprocal(rT[32 * h:32 * h + 1, :], ys[h][64:65, :])
                brec_ps = ps.tile([128, TCH], F32, tag="gen", bufs=2, name=f"brp{c}")
                nc.tensor.matmul(brec_ps[:], pat_t[:], rT[:], start=True, stop=True)
                brec = work.tile([128, TCH], BF16, tag="brec", name=f"brec{c}")
                nc.scalar.copy(brec[:], brec_ps[:])
                ysb = work.tile([128, TCH], BF16, tag="ysb", name=f"ysb{c}")
                for h in range(HPC):
                    nc.vector.tensor_mul(ysb[64 * h:64 * h + 64, :],
                                         ys[h][0:64, :],
                                         brec[64 * h:64 * h + 64, :])
                prev_outproj = outproj_ops(c, ysb)
                if debug:
                    nc.sync.dma_start(out=ysb_d[c], in_=ysb[:])
                    nc.sync.dma_start(out=brec_d[c], in_=brec[:])
                    for h in range(HPC):
                        nc.sync.dma_start(out=rr_d[c, h:h + 1, :],
                                          in_=rT[32 * h:32 * h + 1, :])

            for op in prev_outproj:
                op()
            if debug:
                nc.sync.dma_start(out=qrot_d, in_=qrot[:])
                nc.sync.dma_start(out=krot_d, in_=krot[:])
                nc.sync.dma_start(out=vnat_d, in_=vnat[:, :, :, :])

    nc.compile()
    return nc


def _host_prep(x, w_qkv):
    bf16 = ml_dtypes.bfloat16
    xT = np.ascontiguousarray(x.reshape(BT, C).T).astype(bf16)  # [C, BT]
    perm = np.concatenate([np.arange(0, D, 2), np.arange(1, D, 2)])
    inv = 1.0 / (10000.0 ** (np.arange(0, D, 2, dtype=np.float64) / D))
    f = np.outer(np.arange(T, dtype=np.float64), inv)  # [T, 32]
    cosT = np.cos(f).T
    sinT = np.sin(f).T
    C128 = np.ascontiguousarray(np.concatenate([cosT] * 4, 0)).astype(bf16)
    S128 = np.ascontiguousarray(np.concatenate([sinT] * 4, 0)).astype(bf16)

    PT = np.zeros((128, 128), dtype=np.float32)
    for hb in (0, 64):
        for i in range(32):
            PT[hb + 32 + i, hb + i] = -1.0
            PT[hb + i, hb + 32 + i] = +1.0
    PT = PT.astype(bf16)

    in_maps = []
    for i in range(NCORES):
        h0, h1 = HPC * i, HPC * i + 1
        wq = np.concatenate([w_qkv[:, h0 * D + perm], w_qkv[:, h1 * D + perm]], 1)
        wk = np.concatenate([w_qkv[:, C + h0 * D + perm], w_qkv[:, C + h1 * D + perm]], 1)
        wv = np.concatenate([w_qkv[:, 2 * C + h0 * D:2 * C + (h0 + 1) * D],
                             w_qkv[:, 2 * C + h1 * D:2 * C + (h1 + 1) * D]], 1)
        pat33 = np.zeros((33, 128), dtype=np.float32)
        pat33[0, 0:64] = 1.0
        pat33[32, 64:128] = 1.0
        in_maps.append({
            "xT": xT, "pat33": pat33.astype(bf16),
            "wq": np.ascontiguousarray(wq).astype(bf16),
            "wk": np.ascontiguousarray(wk).astype(bf16),
            "wv": np.ascontiguousarray(wv).astype(bf16),
            "cosT": C128, "sinT": S128, "PT": PT,
        })
    return in_maps


_CACHE = {}


def _get_kernel():
    if "k" not in _CACHE:
        _CACHE["k"] = build_kernel()
    return _CACHE["k"]


def run(x, w_qkv, w_proj, b_proj, trace=False, tmpdirs=(None,), debug=False):
    if debug:
        ncb = build_kernel(debug=True)
    else:
        ncb = _get_kernel()
    x = np.asarray(x)
    w_qkv = np.asarray(w_qkv)
    w_proj = np.asarray(w_proj)
    b_proj = np.asarray(b_proj)
    in_maps = _host_prep(x, w_qkv)
    bf16 = ml_dtypes.bfloat16
    for i in range(NCORES):
        h0 = HPC * i
        wp = np.ascontiguousarray(w_proj[h0 * D:(h0 + HPC) * D]).astype(bf16)
        in_maps[i]["wp"] = wp
    res = run_bass_kernel_spmd(ncb, in_maps, list(range(NCORES)),
                               trace=trace, tmpdir=tmpdirs[0])
    out = np.zeros((BT, C), dtype=np.float32)
    for i in range(NCORES):
        out += res.results[i]["partial"]
    out += b_proj[None, :]
    return out.reshape(B, T, C), res


def kernel(x, w_qkv, w_proj, b_proj):
    out, _ = run(x, w_qkv, w_proj, b_proj)
    return out


# revision 25
# speedup vs baseline: 1.5584x; 1.0924x over previous
"""Causal self-attention (RoPE) Trainium2 kernel, 8-core SPMD, single launch.

Sharding: tensor-parallel over heads. Core i owns heads (2i, 2i+1):
  - qkv projection slice (bf16), RoPE, causal attention, and a PARTIAL output
    projection over its own 128 y-features. Host sums the 8 partials and adds
    the bias (both linear, so they commute with the per-core split).

Design notes:
  - All matmuls are bf16 (fp32r runs in 4-cycle/row HIGH mode on HW).
  - q/k kept feature-major [dim, tok]; head dims permuted to [evens|odds] so
    RoPE rotate-half becomes a signed 32-row block swap, done on the PE with a
    small permutation matmul (PT) instead of SBUF->SBUF DMAs.
  - v computed directly token-major [tok, dim] (x-tile as lhsT), with a ones
    column appended per head so the AV matmul also produces softmax
    denominators for free (row 64 of ys).
  - Scores are [k, q] with the two heads' 512-wide strips packed side by side
    in one 2-bank PSUM strip -> ONE wide exp instruction per (kt, window) on
    the Activation engine (its throughput is the second roofline after PE).
  - Causal masking of diagonal blocks: Pool-engine affine_select zeroing the
    upper triangle of probs (exp is computed unmasked; scores ~ N(0,1)).
  - Normalization: reciprocal of the denominator rows, broadcast across the
    128 feature partitions with a tiny PE matmul (pat2), applied to y during
    the PSUM->SBUF copy, BEFORE the output projection (scale commutes).
  - Projection/out-projection matmuls are interleaved into the attention kt
    loop as "filler" so the PE never head-of-line blocks on exp.
"""

import math
from collections import deque

import numpy as np
import ml_dtypes

import concourse.bass as bass
import concourse.tile as tile
from concourse import bacc, mybir
from concourse.bass_utils import run_bass_kernel_spmd
from concourse.masks import make_identity

F32 = mybir.dt.float32
BF16 = mybir.dt.bfloat16

B, T, C = 2, 2048, 1024
H, D = 16, 64
NCORES = 8
HPC = H // NCORES          # heads per core (2)
BT = B * T                 # 4096 tokens
TCH = 512                  # token chunk = q window
NKT = C // 128             # contraction tiles over C (8)
NCH = BT // TCH            # chunks (8)
KPW = TCH // 128           # k-tiles per window step (4)


def build_kernel(debug=False):
    nc = bacc.Bacc("TRN2", target_bir_lowering=False, debug=False,
                   num_devices=NCORES)
    xT_d = nc.dram_tensor("xT", [C, BT], BF16, kind="ExternalInput").ap()
    wq_d = nc.dram_tensor("wq", [C, 128], BF16, kind="ExternalInput").ap()
    wk_d = nc.dram_tensor("wk", [C, 128], BF16, kind="ExternalInput").ap()
    wv_d = nc.dram_tensor("wv", [C, 128], BF16, kind="ExternalInput").ap()
    cos_d = nc.dram_tensor("cosT", [128, T], BF16, kind="ExternalInput").ap()
    sin_d = nc.dram_tensor("sinT", [128, T], BF16, kind="ExternalInput").ap()
    pt_d = nc.dram_tensor("PT", [128, 128], BF16, kind="ExternalInput").ap()
    pat_d = nc.dram_tensor("pat33", [33, 128], BF16, kind="ExternalInput").ap()
    wp_d = nc.dram_tensor("wp", [128, C], BF16, kind="ExternalInput").ap()
    out_d = nc.dram_tensor("partial", [BT, C], BF16, kind="ExternalOutput").ap()
    if debug:
        qrot_d = nc.dram_tensor("qrot_d", [128, BT], BF16, kind="ExternalOutput").ap()
        krot_d = nc.dram_tensor("krot_d", [128, BT], BF16, kind="ExternalOutput").ap()
        vnat_d = nc.dram_tensor("vnat_d", [128, BT // 128, HPC, 65], BF16,
                                kind="ExternalOutput").ap()
        ysb_d = nc.dram_tensor("ysb_d", [NCH, 128, TCH], BF16, kind="ExternalOutput").ap()
        rr_d = nc.dram_tensor("rr_d", [NCH, HPC, TCH], BF16, kind="ExternalOutput").ap()
        pr_d = nc.dram_tensor("pr_d", [16, 128, 2, TCH], BF16, kind="ExternalOutput").ap()
        brec_d = nc.dram_tensor("brec_d", [NCH, 128, TCH], BF16, kind="ExternalOutput").ap()

    with tile.TileContext(nc) as tc:
        with tc.tile_pool(name="consts", bufs=1) as consts, \
             tc.tile_pool(name="persist", bufs=1) as persist, \
             tc.tile_pool(name="xinp", bufs=2) as xinp, \
             tc.tile_pool(name="work", bufs=2) as work, \
             tc.tile_pool(name="probsp", bufs=3) as probsp, \
             tc.tile_pool(name="ps", bufs=1, space="PSUM") as ps:

            # ---------------- constants ----------------
            wq_t = consts.tile([128, NKT, 128], BF16, tag="wq")
            wk_t = consts.tile([128, NKT, 128], BF16, tag="wk")
            wv_t = consts.tile([128, NKT, 128], BF16, tag="wv")
            nc.sync.dma_start(out=wq_t[:], in_=wq_d.rearrange("(kt p) c -> p kt c", p=128))
            nc.sync.dma_start(out=wk_t[:], in_=wk_d.rearrange("(kt p) c -> p kt c", p=128))
            nc.sync.dma_start(out=wv_t[:], in_=wv_d.rearrange("(kt p) c -> p kt c", p=128))
            cos_t = consts.tile([128, T], BF16, tag="cos")
            sin_t = consts.tile([128, T], BF16, tag="sin")
            nc.sync.dma_start(out=cos_t[:], in_=cos_d)
            nc.sync.dma_start(out=sin_t[:], in_=sin_d)
            pt_t = consts.tile([128, 128], BF16, tag="pt")
            nc.sync.dma_start(out=pt_t[:], in_=pt_d)
            pat_t = consts.tile([33, 128], BF16, tag="pat")
            nc.sync.dma_start(out=pat_t[:], in_=pat_d)
            wp_t = consts.tile([128, C], BF16, tag="wp")
            nc.sync.dma_start(out=wp_t[:], in_=wp_d)
            ident = consts.tile([128, 128], BF16, tag="ident")
            make_identity(nc, ident)

            # ---------------- persistent ----------------
            qrot = persist.tile([128, BT], BF16, tag="qrot")
            krot = persist.tile([128, BT], BF16, tag="krot")
            # v token-major: [tok, ktile, head, dim|one]
            vnat = persist.tile([128, BT // 128, HPC, 65], BF16, tag="vnat")
            nc.gpsimd.memset(vnat[:, :, :, 64:65], 1.0)
            # denominator reciprocals land on rows 0 and 32 (32-aligned engine
            # bases); other rows stay zero so the pat33 broadcast matmul
            # (contraction 33) reproduces each half from its row.
            rT = persist.tile([33, TCH], BF16, tag="rT")
            nc.gpsimd.memset(rT[:, :], 0.0)

            scale = float(1.0 / math.sqrt(D))

            # ---------------- helpers ----------------
            def proj_ops(c):
                """Filler closures computing qkv projection + RoPE of chunk c."""
                g0 = c * TCH
                t0 = g0 % T
                ops = []

                def dma_x():
                    xt = xinp.tile([128, NKT, TCH], BF16, tag="xt", name=f"xt{c}")
                    nc.sync.dma_start(
                        out=xt[:],
                        in_=xT_d.rearrange("(kt p) t -> p kt t", p=128)[:, :, g0:g0 + TCH])
                    return xt
                xt_box = {}
                ops.append(lambda: xt_box.__setitem__("t", dma_x()))

                def qk_mm(wt, half, box, tag):
                    def f():
                        if half == 0:
                            box["ps"] = ps.tile([128, TCH], F32, tag="gen", bufs=2,
                                                name=f"{tag}ps{c}")
                        pp = box["ps"]
                        for kt in range(4 * half, 4 * half + 4):
                            nc.tensor.matmul(pp[:], wt[:, kt, :], xt_box["t"][:, kt, :],
                                             start=(kt == 0), stop=(kt == 7))
                    return f

                def qk_rope(box, dest, tag):
                    def f():
                        raw = work.tile([128, TCH], BF16, tag="raw", name=f"raw{tag}{c}")
                        nc.scalar.copy(raw[:], box["ps"][:])
                        sh = ps.tile([128, TCH], F32, tag="gen", bufs=2, name=f"sh{tag}{c}")
                        nc.tensor.matmul(sh[:], pt_t[:], raw[:], start=True, stop=True)
                        tmp = work.tile([128, TCH], BF16, tag="tmp", name=f"tmp{tag}{c}")
                        nc.vector.tensor_mul(tmp[:], sh[:], sin_t[:, t0:t0 + TCH])
                        dst = dest[:, g0:g0 + TCH]
                        nc.vector.tensor_mul(dst, raw[:], cos_t[:, t0:t0 + TCH])
                        nc.vector.tensor_add(dst, dst, tmp[:])
                    return f

                # NOTE: every "gen"-ring PSUM tile must be consumed before two
                # more gen allocations happen (ring bufs=2) — keep each
                # producer's reader within the next closure.
                qb, kb = {}, {}
                ops.append(qk_mm(wq_t, 0, qb, "q"))
                ops.append(qk_mm(wq_t, 1, qb, "q"))
                ops.append(qk_rope(qb, qrot, "q"))
                ops.append(qk_mm(wk_t, 0, kb, "k"))
                ops.append(qk_mm(wk_t, 1, kb, "k"))

                vb = {}

                def v_mm(half):
                    def f():
                        if half == 0:
                            vb["ps"] = ps.tile([128, TCH], F32, tag="gen", bufs=2,
                                               name=f"vps{c}")
                        pp = vb["ps"]
                        for kt in range(4 * half, 4 * half + 4):
                            nc.tensor.matmul(pp[:], wv_t[:, kt, :], xt_box["t"][:, kt, :],
                                             start=(kt == 0), stop=(kt == 7))
                    return f

                def v_stage():
                    vstage = work.tile([128, TCH], BF16, tag="vstage", name=f"vst{c}")
                    nc.vector.tensor_copy(vstage[:], vb["ps"][:])
                    vb["st"] = vstage

                def v_tr(m):
                    def f():
                        trp = ps.tile([128, 128], BF16, tag="gen", bufs=2,
                                      name=f"vtr{c}_{m}")
                        nc.tensor.transpose(trp[:], vb["st"][:, 128 * m:128 * (m + 1)],
                                            ident[:])
                        vt = c * KPW + m
                        for h in range(HPC):
                            nc.vector.tensor_copy(vnat[:, vt, h, 0:64],
                                                  trp[:, 64 * h:64 * h + 64])
                    return f

                ops.append(qk_rope(kb, krot, "k"))
                ops.append(v_mm(0))
                ops.append(v_mm(1))
                ops.append(v_stage)
                ops.append(v_tr(0))
                ops.append(v_tr(1))
                ops.append(v_tr(2))
                ops.append(v_tr(3))
                return ops

            def outproj_ops(c, ysb):
                """Filler closures projecting normalized y chunk c through wp."""
                g0 = c * TCH
                ops = []

                def otile(m):
                    def f():
                        for n in range(2):
                            po = ps.tile([128, TCH], F32, tag="gen", bufs=2,
                                         name=f"po{c}_{m}_{n}")
                            nc.tensor.matmul(po[:], ysb[:, 128 * m:128 * (m + 1)],
                                             wp_t[:, TCH * n:TCH * (n + 1)],
                                             start=True, stop=True)
                            ostage = work.tile([128, TCH], BF16, tag="ostage",
                                               bufs=3, name=f"os{c}_{m}_{n}")
                            nc.vector.tensor_copy(ostage[:], po[:])
                            nc.sync.dma_start(
                                out=out_d[g0 + 128 * m:g0 + 128 * (m + 1),
                                          TCH * n:TCH * (n + 1)],
                                in_=ostage[:])
                    return f
                for m in range(4):
                    ops.append(otile(m))
                return ops

            # ---------------- main pipeline ----------------
            queue = deque(proj_ops(0))
            while queue:  # chunk 0 projection up-front
                queue.popleft()()

            prev_outproj = []
            for c in range(NCH):
                b, wl = divmod(c, T // TCH)
                q0l = wl * TCH
                gq = c * TCH
                nkt = KPW * (wl + 1)

                if c + 1 < NCH:
                    queue.extend(proj_ops(c + 1))
                queue.extend(prev_outproj)
                prev_outproj = []

                ys = [ps.tile([65, TCH], F32, tag=f"ys{h}", bufs=1, name=f"ys{h}_{c}")
                      for h in range(HPC)]
                strips = {}
                probs = {}

                def sc(kt):
                    k0l = 128 * kt
                    js = max(k0l - q0l, 0)
                    st = ps.tile([128, 2, TCH], F32, tag="strip", bufs=2,
                                 name=f"st{c}_{kt}")
                    strips[kt] = st
                    for h in range(HPC):
                        hp = 64 * h
                        nc.tensor.matmul(
                            st[:, h, js:TCH],
                            krot[hp:hp + 64, b * T + k0l:b * T + k0l + 128],
                            qrot[hp:hp + 64, gq + js:gq + TCH],
                            start=True, stop=True, tile_position=(hp, 0))
                    pr = probsp.tile([128, 2, TCH], BF16, tag="pr", name=f"pr{c}_{kt}")
                    probs[kt] = pr
                    nc.scalar.activation(pr[:, :, js:TCH], st[:, :, js:TCH],
                                         mybir.ActivationFunctionType.Exp,
                                         scale=scale)
                    if js > 0 or kt == q0l // 128:
                        # diagonal block: zero probs where q < k, i.e. keep
                        # j - p >= 0 (p = key partition, j = query column)
                        nc.gpsimd.affine_select(
                            out=pr[:, :, js:js + 128], in_=pr[:, :, js:js + 128],
                            compare_op=mybir.AluOpType.is_ge, fill=0.0,
                            base=0, pattern=[[0, 2], [1, 128]],
                            channel_multiplier=-1)
                    if debug and c == NCH - 1:
                        nc.sync.dma_start(out=pr_d[kt], in_=pr[:, :, :])

                def av(kt):
                    k0l = 128 * kt
                    js = max(k0l - q0l, 0)
                    vt = b * (T // 128) + kt
                    for h in range(HPC):
                        nc.tensor.matmul(ys[h][:, js:TCH],
                                         vnat[:, vt, h, :],
                                         probs[kt][:, h, js:TCH],
                                         start=(kt == 0), stop=(kt == nkt - 1))
                    del probs[kt], strips[kt]

                sc(0)
                if nkt > 1:
                    sc(1)
                for kt in range(nkt):
                    av(kt)
                    ndrain = -(-len(queue) // (nkt - kt)) if kt < nkt else 0
                    for _ in range(min(ndrain, len(queue))):
                        queue.popleft()()
                    if kt + 2 < nkt:
                        sc(kt + 2)

                # normalization: ysb = ys/denom (bf16), feature-major
                # 1/d computed as exp(-ln(d)) on Act: Ln and Exp share one
                # activation table, and Act reads PSUM directly.
                for h in range(HPC):
                    lnd = work.tile([1, TCH], F32, tag=f"ln{h}", name=f"ln{h}_{c}")
                    nc.scalar.activation(lnd[0:1, :], ys[h][64:65, :],
                                         mybir.ActivationFunctionType.Ln)
                    nc.scalar.activation(rT[32 * h:32 * h + 1, :], lnd[0:1, :],
                                         mybir.ActivationFunctionType.Exp,
                                         scale=-1.0)
                brec_ps = ps.tile([128, TCH], F32, tag="gen", bufs=2, name=f"brp{c}")
                nc.tensor.matmul(brec_ps[:], pat_t[:], rT[:], start=True, stop=True)
                brec = work.tile([128, TCH], BF16, tag="brec", name=f"brec{c}")
                nc.scalar.copy(brec[:], brec_ps[:])
                ysb = work.tile([128, TCH], BF16, tag="ysb", name=f"ysb{c}")
                for h in range(HPC):
                    nc.vector.tensor_mul(ysb[64 * h:64 * h + 64, :],
                                         ys[h][0:64, :],
                                         brec[64 * h:64 * h + 64, :])
                prev_outproj = outproj_ops(c, ysb)
                if debug:
                    nc.sync.dma_start(out=ysb_d[c], in_=ysb[:])
                    nc.sync.dma_start(out=brec_d[c], in_=brec[:])
                    for h in range(HPC):
                        nc.sync.dma_start(out=rr_d[c, h:h + 1, :],
                                          in_=rT[32 * h:32 * h + 1, :])

            for op in prev_outproj:
                op()
            if debug:
                nc.sync.dma_start(out=qrot_d, in_=qrot[:])
                nc.sync.dma_start(out=krot_d, in_=krot[:])
                nc.sync.dma_start(out=vnat_d, in_=vnat[:, :, :, :])

    nc.compile()
    return nc


def _host_prep(x, w_qkv):
    bf16 = ml_dtypes.bfloat16
    xT = np.ascontiguousarray(x.reshape(BT, C).T).astype(bf16)  # [C, BT]
    perm = np.concatenate([np.arange(0, D, 2), np.arange(1, D, 2)])
    inv = 1.0 / (10000.0 ** (np.arange(0, D, 2, dtype=np.float64) / D))
    f = np.outer(np.arange(T, dtype=np.float64), inv)  # [T, 32]
    cosT = np.cos(f).T
    sinT = np.sin(f).T
    C128 = np.ascontiguousarray(np.concatenate([cosT] * 4, 0)).astype(bf16)
    S128 = np.ascontiguousarray(np.concatenate([sinT] * 4, 0)).astype(bf16)

    PT = np.zeros((128, 128), dtype=np.float32)
    for hb in (0, 64):
        for i in range(32):
            PT[hb + 32 + i, hb + i] = -1.0
            PT[hb + i, hb + 32 + i] = +1.0
    PT = PT.astype(bf16)

    in_maps = []
    for i in range(NCORES):
        h0, h1 = HPC * i, HPC * i + 1
        wq = np.concatenate([w_qkv[:, h0 * D + perm], w_qkv[:, h1 * D + perm]], 1)
        wk = np.concatenate([w_qkv[:, C + h0 * D + perm], w_qkv[:, C + h1 * D + perm]], 1)
        wv = np.concatenate([w_qkv[:, 2 * C + h0 * D:2 * C + (h0 + 1) * D],
                             w_qkv[:, 2 * C + h1 * D:2 * C + (h1 + 1) * D]], 1)
        pat33 = np.zeros((33, 128), dtype=np.float32)
        pat33[0, 0:64] = 1.0
        pat33[32, 64:128] = 1.0
        in_maps.append({
            "xT": xT, "pat33": pat33.astype(bf16),
            "wq": np.ascontiguousarray(wq).astype(bf16),
            "wk": np.ascontiguousarray(wk).astype(bf16),
            "wv": np.ascontiguousarray(wv).astype(bf16),
            "cosT": C128, "sinT": S128, "PT": PT,
        })
    return in_maps


_CACHE = {}


def _get_kernel():
    if "k" not in _CACHE:
        _CACHE["k"] = build_kernel()
    return _CACHE["k"]


def run(x, w_qkv, w_proj, b_proj, trace=False, tmpdirs=(None,), debug=False):
    if debug:
        ncb = build_kernel(debug=True)
    else:
        ncb = _get_kernel()
    x = np.asarray(x)
    w_qkv = np.asarray(w_qkv)
    w_proj = np.asarray(w_proj)
    b_proj = np.asarray(b_proj)
    in_maps = _host_prep(x, w_qkv)
    bf16 = ml_dtypes.bfloat16
    for i in range(NCORES):
        h0 = HPC * i
        wp = np.ascontiguousarray(w_proj[h0 * D:(h0 + HPC) * D]).astype(bf16)
        in_maps[i]["wp"] = wp
    res = run_bass_kernel_spmd(ncb, in_maps, list(range(NCORES)),
                               trace=trace, tmpdir=tmpdirs[0])
    out = np.zeros((BT, C), dtype=np.float32)
    for i in range(NCORES):
        out += res.results[i]["partial"]
    out += b_proj[None, :]
    return out.reshape(B, T, C), res


def kernel(x, w_qkv, w_proj, b_proj):
    out, _ = run(x, w_qkv, w_proj, b_proj)
    return out


# revision 26
# speedup vs baseline: 1.5751x; 1.0107x over previous
"""Causal self-attention (RoPE) Trainium2 kernel, 8-core SPMD, single launch.

Sharding: tensor-parallel over heads. Core i owns heads (2i, 2i+1):
  - qkv projection slice (bf16), RoPE, causal attention, and a PARTIAL output
    projection over its own 128 y-features. Host sums the 8 partials and adds
    the bias (both linear, so they commute with the per-core split).

Design notes:
  - All matmuls are bf16 (fp32r runs in 4-cycle/row HIGH mode on HW).
  - q/k kept feature-major [dim, tok]; head dims permuted to [evens|odds] so
    RoPE rotate-half becomes a signed 32-row block swap, done on the PE with a
    small permutation matmul (PT) instead of SBUF->SBUF DMAs.
  - v computed directly token-major [tok, dim] (x-tile as lhsT), with a ones
    column appended per head so the AV matmul also produces softmax
    denominators for free (row 64 of ys).
  - Scores are [k, q] with the two heads' 512-wide strips packed side by side
    in one 2-bank PSUM strip -> ONE wide exp instruction per (kt, window) on
    the Activation engine (its throughput is the second roofline after PE).
  - Causal masking of diagonal blocks: Pool-engine affine_select zeroing the
    upper triangle of probs (exp is computed unmasked; scores ~ N(0,1)).
  - Normalization: reciprocal of the denominator rows, broadcast across the
    128 feature partitions with a tiny PE matmul (pat2), applied to y during
    the PSUM->SBUF copy, BEFORE the output projection (scale commutes).
  - Projection/out-projection matmuls are interleaved into the attention kt
    loop as "filler" so the PE never head-of-line blocks on exp.
"""

import math
from collections import deque

import numpy as np
import ml_dtypes

import concourse.bass as bass
import concourse.tile as tile
from concourse import bacc, mybir
from concourse.bass_utils import run_bass_kernel_spmd
from concourse.masks import make_identity

F32 = mybir.dt.float32
BF16 = mybir.dt.bfloat16

B, T, C = 2, 2048, 1024
H, D = 16, 64
NCORES = 8
HPC = H // NCORES          # heads per core (2)
BT = B * T                 # 4096 tokens
TCH = 512                  # token chunk = q window
NKT = C // 128             # contraction tiles over C (8)
NCH = BT // TCH            # chunks (8)
KPW = TCH // 128           # k-tiles per window step (4)


def build_kernel(debug=False):
    nc = bacc.Bacc("TRN2", target_bir_lowering=False, debug=False,
                   num_devices=NCORES)
    xT_d = nc.dram_tensor("xT", [C, BT], BF16, kind="ExternalInput").ap()
    wq_d = nc.dram_tensor("wq", [C, 128], BF16, kind="ExternalInput").ap()
    wk_d = nc.dram_tensor("wk", [C, 128], BF16, kind="ExternalInput").ap()
    wv_d = nc.dram_tensor("wv", [C, 128], BF16, kind="ExternalInput").ap()
    cos_d = nc.dram_tensor("cosT", [128, T], BF16, kind="ExternalInput").ap()
    sin_d = nc.dram_tensor("sinT", [128, T], BF16, kind="ExternalInput").ap()
    pt_d = nc.dram_tensor("PT", [128, 128], BF16, kind="ExternalInput").ap()
    pat_d = nc.dram_tensor("pat33", [33, 128], BF16, kind="ExternalInput").ap()
    wp_d = nc.dram_tensor("wp", [128, C], BF16, kind="ExternalInput").ap()
    out_d = nc.dram_tensor("partial", [BT, C], BF16, kind="ExternalOutput").ap()
    if debug:
        qrot_d = nc.dram_tensor("qrot_d", [128, BT], BF16, kind="ExternalOutput").ap()
        krot_d = nc.dram_tensor("krot_d", [128, BT], BF16, kind="ExternalOutput").ap()
        vnat_d = nc.dram_tensor("vnat_d", [128, BT // 128, HPC, 65], BF16,
                                kind="ExternalOutput").ap()
        ysb_d = nc.dram_tensor("ysb_d", [NCH, 128, TCH], BF16, kind="ExternalOutput").ap()
        rr_d = nc.dram_tensor("rr_d", [NCH, HPC, TCH], BF16, kind="ExternalOutput").ap()
        pr_d = nc.dram_tensor("pr_d", [16, 128, 2, TCH], BF16, kind="ExternalOutput").ap()
        brec_d = nc.dram_tensor("brec_d", [NCH, 128, TCH], BF16, kind="ExternalOutput").ap()

    with tile.TileContext(nc) as tc:
        with tc.tile_pool(name="consts", bufs=1) as consts, \
             tc.tile_pool(name="persist", bufs=1) as persist, \
             tc.tile_pool(name="xinp", bufs=2) as xinp, \
             tc.tile_pool(name="work", bufs=2) as work, \
             tc.tile_pool(name="probsp", bufs=3) as probsp, \
             tc.tile_pool(name="ps", bufs=1, space="PSUM") as ps:

            # ---------------- constants ----------------
            wq_t = consts.tile([128, NKT, 128], BF16, tag="wq")
            wk_t = consts.tile([128, NKT, 128], BF16, tag="wk")
            wv_t = consts.tile([128, NKT, 128], BF16, tag="wv")
            nc.sync.dma_start(out=wq_t[:], in_=wq_d.rearrange("(kt p) c -> p kt c", p=128))
            nc.sync.dma_start(out=wk_t[:], in_=wk_d.rearrange("(kt p) c -> p kt c", p=128))
            nc.sync.dma_start(out=wv_t[:], in_=wv_d.rearrange("(kt p) c -> p kt c", p=128))
            cos_t = consts.tile([128, T], BF16, tag="cos")
            sin_t = consts.tile([128, T], BF16, tag="sin")
            nc.sync.dma_start(out=cos_t[:], in_=cos_d)
            nc.sync.dma_start(out=sin_t[:], in_=sin_d)
            pt_t = consts.tile([128, 128], BF16, tag="pt")
            nc.sync.dma_start(out=pt_t[:], in_=pt_d)
            pat_t = consts.tile([33, 128], BF16, tag="pat")
            nc.sync.dma_start(out=pat_t[:], in_=pat_d)
            wp_t = consts.tile([128, C], BF16, tag="wp")
            nc.sync.dma_start(out=wp_t[:], in_=wp_d)
            ident = consts.tile([128, 128], BF16, tag="ident")
            make_identity(nc, ident)

            # ---------------- persistent ----------------
            qrot = persist.tile([128, BT], BF16, tag="qrot")
            krot = persist.tile([128, BT], BF16, tag="krot")
            # v token-major: [tok, ktile, head, dim|one]
            vnat = persist.tile([128, BT // 128, HPC, 65], BF16, tag="vnat")
            nc.gpsimd.memset(vnat[:, :, :, 64:65], 1.0)
            # denominator reciprocals land on rows 0 and 32 (32-aligned engine
            # bases); other rows stay zero so the pat33 broadcast matmul
            # (contraction 33) reproduces each half from its row.
            rT = persist.tile([33, TCH], BF16, tag="rT")
            nc.gpsimd.memset(rT[:, :], 0.0)

            scale = float(1.0 / math.sqrt(D))

            # ---------------- helpers ----------------
            def proj_ops(c):
                """Filler closures computing qkv projection + RoPE of chunk c."""
                g0 = c * TCH
                t0 = g0 % T
                ops = []

                def dma_x():
                    xt = xinp.tile([128, NKT, TCH], BF16, tag="xt", name=f"xt{c}")
                    nc.sync.dma_start(
                        out=xt[:],
                        in_=xT_d.rearrange("(kt p) t -> p kt t", p=128)[:, :, g0:g0 + TCH])
                    return xt
                xt_box = {}
                ops.append(lambda: xt_box.__setitem__("t", dma_x()))

                def qk_mm(wt, half, box, tag):
                    def f():
                        if half == 0:
                            box["ps"] = ps.tile([128, TCH], F32, tag="gen", bufs=2,
                                                name=f"{tag}ps{c}")
                        pp = box["ps"]
                        for kt in range(4 * half, 4 * half + 4):
                            nc.tensor.matmul(pp[:], wt[:, kt, :], xt_box["t"][:, kt, :],
                                             start=(kt == 0), stop=(kt == 7))
                    return f

                def qk_rope(box, dest, tag):
                    def f():
                        raw = work.tile([128, TCH], BF16, tag="raw", name=f"raw{tag}{c}")
                        nc.scalar.copy(raw[:], box["ps"][:])
                        sh = ps.tile([128, TCH], F32, tag="gen", bufs=2, name=f"sh{tag}{c}")
                        nc.tensor.matmul(sh[:], pt_t[:], raw[:], start=True, stop=True)
                        tmp = work.tile([128, TCH], BF16, tag="tmp", name=f"tmp{tag}{c}")
                        nc.vector.tensor_mul(tmp[:], sh[:], sin_t[:, t0:t0 + TCH])
                        dst = dest[:, g0:g0 + TCH]
                        nc.vector.tensor_mul(dst, raw[:], cos_t[:, t0:t0 + TCH])
                        nc.vector.tensor_add(dst, dst, tmp[:])
                    return f

                # NOTE: every "gen"-ring PSUM tile must be consumed before two
                # more gen allocations happen (ring bufs=2) — keep each
                # producer's reader within the next closure.
                qb, kb = {}, {}
                ops.append(qk_mm(wq_t, 0, qb, "q"))
                ops.append(qk_mm(wq_t, 1, qb, "q"))
                ops.append(qk_rope(qb, qrot, "q"))
                ops.append(qk_mm(wk_t, 0, kb, "k"))
                ops.append(qk_mm(wk_t, 1, kb, "k"))

                vb = {}

                def v_mm(half):
                    def f():
                        if half == 0:
                            vb["ps"] = ps.tile([128, TCH], F32, tag="gen", bufs=2,
                                               name=f"vps{c}")
                        pp = vb["ps"]
                        for kt in range(4 * half, 4 * half + 4):
                            nc.tensor.matmul(pp[:], wv_t[:, kt, :], xt_box["t"][:, kt, :],
                                             start=(kt == 0), stop=(kt == 7))
                    return f

                def v_stage():
                    vstage = work.tile([128, TCH], BF16, tag="vstage", name=f"vst{c}")
                    nc.vector.tensor_copy(vstage[:], vb["ps"][:])
                    vb["st"] = vstage

                def v_tr(m):
                    def f():
                        trp = ps.tile([128, 128], BF16, tag="gen", bufs=2,
                                      name=f"vtr{c}_{m}")
                        nc.tensor.transpose(trp[:], vb["st"][:, 128 * m:128 * (m + 1)],
                                            ident[:])
                        vt = c * KPW + m
                        for h in range(HPC):
                            nc.vector.tensor_copy(vnat[:, vt, h, 0:64],
                                                  trp[:, 64 * h:64 * h + 64])
                    return f

                ops.append(qk_rope(kb, krot, "k"))
                ops.append(v_mm(0))
                ops.append(v_mm(1))
                ops.append(v_stage)
                ops.append(v_tr(0))
                ops.append(v_tr(1))
                ops.append(v_tr(2))
                ops.append(v_tr(3))
                return ops

            def outproj_ops(c, ybox):
                """Filler closures projecting normalized y chunk c through wp."""
                g0 = c * TCH
                ops = []

                def otile(m):
                    def f():
                        ysb = ybox["ysb"]
                        for n in range(2):
                            po = ps.tile([128, TCH], F32, tag="gen", bufs=2,
                                         name=f"po{c}_{m}_{n}")
                            nc.tensor.matmul(po[:], ysb[:, 128 * m:128 * (m + 1)],
                                             wp_t[:, TCH * n:TCH * (n + 1)],
                                             start=True, stop=True)
                            ostage = work.tile([128, TCH], BF16, tag="ostage",
                                               bufs=3, name=f"os{c}_{m}_{n}")
                            nc.vector.tensor_copy(ostage[:], po[:])
                            nc.sync.dma_start(
                                out=out_d[g0 + 128 * m:g0 + 128 * (m + 1),
                                          TCH * n:TCH * (n + 1)],
                                in_=ostage[:])
                    return f
                for m in range(4):
                    ops.append(otile(m))
                return ops

            # ---------------- main pipeline ----------------
            queue = deque(proj_ops(0))
            while queue:  # chunk 0 projection up-front
                queue.popleft()()

            prev_outproj = []
            for c in range(NCH):
                b, wl = divmod(c, T // TCH)
                q0l = wl * TCH
                gq = c * TCH
                nkt = KPW * (wl + 1)

                if c + 1 < NCH:
                    queue.extend(proj_ops(c + 1))
                queue.extend(prev_outproj)
                prev_outproj = []

                ys = [ps.tile([65, TCH], F32, tag=f"ys{h}", bufs=1, name=f"ys{h}_{c}")
                      for h in range(HPC)]
                strips = {}
                probs = {}

                def sc(kt):
                    k0l = 128 * kt
                    js = max(k0l - q0l, 0)
                    st = ps.tile([128, 2, TCH], F32, tag="strip", bufs=2,
                                 name=f"st{c}_{kt}")
                    strips[kt] = st
                    for h in range(HPC):
                        hp = 64 * h
                        nc.tensor.matmul(
                            st[:, h, js:TCH],
                            krot[hp:hp + 64, b * T + k0l:b * T + k0l + 128],
                            qrot[hp:hp + 64, gq + js:gq + TCH],
                            start=True, stop=True, tile_position=(hp, 0))
                    pr = probsp.tile([128, 2, TCH], BF16, tag="pr", name=f"pr{c}_{kt}")
                    probs[kt] = pr
                    nc.scalar.activation(pr[:, :, js:TCH], st[:, :, js:TCH],
                                         mybir.ActivationFunctionType.Exp,
                                         scale=scale)
                    if js > 0 or kt == q0l // 128:
                        # diagonal block: zero probs where q < k, i.e. keep
                        # j - p >= 0 (p = key partition, j = query column)
                        nc.gpsimd.affine_select(
                            out=pr[:, :, js:js + 128], in_=pr[:, :, js:js + 128],
                            compare_op=mybir.AluOpType.is_ge, fill=0.0,
                            base=0, pattern=[[0, 2], [1, 128]],
                            channel_multiplier=-1)
                    if debug and c == NCH - 1:
                        nc.sync.dma_start(out=pr_d[kt], in_=pr[:, :, :])

                def av(kt):
                    k0l = 128 * kt
                    js = max(k0l - q0l, 0)
                    vt = b * (T // 128) + kt
                    for h in range(HPC):
                        nc.tensor.matmul(ys[h][:, js:TCH],
                                         vnat[:, vt, h, :],
                                         probs[kt][:, h, js:TCH],
                                         start=(kt == 0), stop=(kt == nkt - 1))
                    del probs[kt], strips[kt]

                sc(0)
                if nkt > 1:
                    sc(1)
                for kt in range(nkt):
                    ndrain = -(-len(queue) // (nkt - kt))
                    for _ in range(min(ndrain, len(queue))):
                        queue.popleft()()
                    av(kt)
                    if kt + 2 < nkt:
                        sc(kt + 2)

                # normalization deferred into the next window's filler queue so
                # the recip chain's latency hides under the next scores/exp.
                def norm_ops(c, ys):
                    box = {}

                    def n1():
                        # 1/d: copy denom rows to SBUF fp32, 1-op DVE approx
                        # reciprocal (18-bit), convert to bf16 rT rows.
                        for h in range(HPC):
                            dsb = work.tile([1, TCH], F32, tag=f"d{h}",
                                            name=f"d{h}_{c}")
                            nc.vector.tensor_copy(dsb[0:1, :], ys[h][64:65, :])
                            rf = work.tile([1, TCH], F32, tag=f"rf{h}",
                                           name=f"rf{h}_{c}")
                            nc.vector.reciprocal_approx_fast(rf[0:1, :], dsb[0:1, :])
                            with nc.allow_low_precision(reason="recip bf16 ok"):
                                nc.vector.tensor_copy(rT[32 * h:32 * h + 1, :],
                                                      rf[0:1, :])
                        if debug:
                            for h in range(HPC):
                                nc.sync.dma_start(out=rr_d[c, h:h + 1, :],
                                                  in_=rT[32 * h:32 * h + 1, :])

                    def n2():
                        brec_ps = ps.tile([128, TCH], F32, tag="gen", bufs=2,
                                          name=f"brp{c}")
                        nc.tensor.matmul(brec_ps[:], pat_t[:], rT[:],
                                         start=True, stop=True)
                        brec = work.tile([128, TCH], BF16, tag="brec",
                                         name=f"brec{c}")
                        nc.scalar.copy(brec[:], brec_ps[:])
                        box["brec"] = brec
                        if debug:
                            nc.sync.dma_start(out=brec_d[c], in_=brec[:])

                    def n3():
                        brec = box["brec"]
                        ysb = work.tile([128, TCH], BF16, tag="ysb",
                                        name=f"ysb{c}")
                        for h in range(HPC):
                            nc.vector.tensor_mul(ysb[64 * h:64 * h + 64, :],
                                                 ys[h][0:64, :],
                                                 brec[64 * h:64 * h + 64, :])
                        box["ysb"] = ysb
                        if debug:
                            nc.sync.dma_start(out=ysb_d[c], in_=ysb[:])
                    return [n1, n2, n3], box

                nops, ybox = norm_ops(c, ys)
                prev_outproj = nops + outproj_ops(c, ybox)

            while queue:
                queue.popleft()()
            for op in prev_outproj:
                op()
            if debug:
                nc.sync.dma_start(out=qrot_d, in_=qrot[:])
                nc.sync.dma_start(out=krot_d, in_=krot[:])
                nc.sync.dma_start(out=vnat_d, in_=vnat[:, :, :, :])

    nc.compile()
    return nc


def _host_prep(x, w_qkv):
    bf16 = ml_dtypes.bfloat16
    xT = np.ascontiguousarray(x.reshape(BT, C).T).astype(bf16)  # [C, BT]
    perm = np.concatenate([np.arange(0, D, 2), np.arange(1, D, 2)])
    inv = 1.0 / (10000.0 ** (np.arange(0, D, 2, dtype=np.float64) / D))
    f = np.outer(np.arange(T, dtype=np.float64), inv)  # [T, 32]
    cosT = np.cos(f).T
    sinT = np.sin(f).T
    C128 = np.ascontiguousarray(np.concatenate([cosT] * 4, 0)).astype(bf16)
    S128 = np.ascontiguousarray(np.concatenate([sinT] * 4, 0)).astype(bf16)

    PT = np.zeros((128, 128), dtype=np.float32)
    for hb in (0, 64):
        for i in range(32):
            PT[hb + 32 + i, hb + i] = -1.0
            PT[hb + i, hb + 32 + i] = +1.0
    PT = PT.astype(bf16)

    in_maps = []
    for i in range(NCORES):
        h0, h1 = HPC * i, HPC * i + 1
        wq = np.concatenate([w_qkv[:, h0 * D + perm], w_qkv[:, h1 * D + perm]], 1)
        wk = np.concatenate([w_qkv[:, C + h0 * D + perm], w_qkv[:, C + h1 * D + perm]], 1)
        wv = np.concatenate([w_qkv[:, 2 * C + h0 * D:2 * C + (h0 + 1) * D],
                             w_qkv[:, 2 * C + h1 * D:2 * C + (h1 + 1) * D]], 1)
        pat33 = np.zeros((33, 128), dtype=np.float32)
        pat33[0, 0:64] = 1.0
        pat33[32, 64:128] = 1.0
        in_maps.append({
            "xT": xT, "pat33": pat33.astype(bf16),
            "wq": np.ascontiguousarray(wq).astype(bf16),
            "wk": np.ascontiguousarray(wk).astype(bf16),
            "wv": np.ascontiguousarray(wv).astype(bf16),
            "cosT": C128, "sinT": S128, "PT": PT,
        })
    return in_maps


_CACHE = {}


def _get_kernel():
    if "k" not in _CACHE:
        _CACHE["k"] = build_kernel()
    return _CACHE["k"]


def run(x, w_qkv, w_proj, b_proj, trace=False, tmpdirs=(None,), debug=False):
    if debug:
        ncb = build_kernel(debug=True)
    else:
        ncb = _get_kernel()
    x = np.asarray(x)
    w_qkv = np.asarray(w_qkv)
    w_proj = np.asarray(w_proj)
    b_proj = np.asarray(b_proj)
    in_maps = _host_prep(x, w_qkv)
    bf16 = ml_dtypes.bfloat16
    for i in range(NCORES):
        h0 = HPC * i
        wp = np.ascontiguousarray(w_proj[h0 * D:(h0 + HPC) * D]).astype(bf16)
        in_maps[i]["wp"] = wp
    res = run_bass_kernel_spmd(ncb, in_maps, list(range(NCORES)),
                               trace=trace, tmpdir=tmpdirs[0])
    out = np.zeros((BT, C), dtype=np.float32)
    for i in range(NCORES):
        out += res.results[i]["partial"]
    out += b_proj[None, :]
    return out.reshape(B, T, C), res


def kernel(x, w_qkv, w_proj, b_proj):
    out, _ = run(x, w_qkv, w_proj, b_proj)
    return out
